# revision 8
# baseline (speedup 1.0000x reference)
"""Trainium2 Bass kernel for nn_Critic (LSTM critic over T=512 steps) — V3.

Sharding: pure data parallel. B=256 batch rows are split across 8 cores
(32 rows each); all weights are replicated. The sequential LSTM scan runs
locally per core.

V3 redesign vs V2: the profiled metric is dominated by per-instruction
overhead (~0.53us/instr: V2 had 60.9k instructions ~= 32.25ms), so V3
minimizes instruction count (~11k) rather than modeled engine cycles.

  * Batch-major scan. z_t is computed as [32 b, 1024 u'] with the small
    recurrent state H^T as the PE *stationary* operand and the big weight
    matrices (Wl, Ul) as the *moving* operand:
        z_t[b,u'] = sum_k x_t[k,b]·Wl[k,u'] + sum_u H[u,b]·Ul[u,u']
    -> 3 stationary loads + 6 matmuls per step instead of V2's 48
    PE instructions (16 gate-block matmuls x Ldweights, x2 sub-batches).
  * The x-projection accumulates directly into the z PSUM banks one step
    ahead of the recurrence (interleaved accumulation groups,
    skip_group_check), so no separate pzx buffer/add is needed.
  * Gates stay in natural [i f g o] column order (contiguous 256-col
    slices of z) — no strided gate interleaving.
  * All-tanh gates + doubled state (H=2h, C2=2c) as V2: sigmoid folded
    into weight scales, cell update = 3 stt + scaled tanh + stt.
  * Hmax amortized: H lands in a 4-slot ring; one reduce_max + one max
    every 4 steps.
  * H^T for the next step via 2 PE transposes + 1 copy per step.

Reference quirks honored (as V2):
  * inp3 = elu(empty @ Woi + boi) = elu(boi) broadcast -> constant; its
    contribution inp3 @ Wl[96:160] is folded into the z bias.
  * osc_state and Woi (shape [0,64]) are unused.
  * only osc[..., :64] is ever read.
"""

import os
import sys

sys.path.insert(0, "/opt/trn_rl_repo")

from contextlib import ExitStack

import numpy as np

import concourse.bass as bass
import concourse.bacc as bacc
import concourse.mybir as mybir
import concourse.tile as tile
from concourse.masks import make_identity

FP32 = mybir.dt.float32
BF16 = mybir.dt.bfloat16
AF = mybir.ActivationFunctionType
ALU = mybir.AluOpType

# Problem dims
B_FULL, T_FULL, A = 256, 512, 32
DM, DR = 64, 128
U = 256                 # lstm units (== combine units)
U4 = 4 * U              # 1024
OSC_HALF = 64
NCORES = 8
B = B_FULL // NCORES    # 32 batch rows per core
XROWS = A + OSC_HALF    # 96 feature rows of x (plus a ones row)
XK = XROWS + 1          # 97

# Weight (Wl/Ul) column layout is [i f g o] (256 each).
# tanh fold: sigmoid(x)=0.5(1+tanh(x/2)) -> scale i/f/o columns by 0.5.
# doubled state: Ul multiplies H=2h -> extra 0.5 on all Ul columns.
WL_SCALE = [0.5, 0.5, 1.0, 0.5]     # per 256-col gate block [i f g o]
UL_SCALE = [0.25, 0.25, 0.5, 0.25]


def _elu(nc, pool, out_ap, y_ap, shape, scale=1.0):
    """out = scale * elu(y) = scale * max(y, exp(min(y, 0)) - 1), exact."""
    m = pool.tile(shape, FP32, tag="elu_m")
    nc.vector.tensor_scalar_min(m, y_ap, 0.0)
    e = pool.tile(shape, FP32, tag="elu_e")
    nc.scalar.activation(e, m, AF.Exp)
    if scale == 1.0:
        nc.vector.scalar_tensor_tensor(out_ap, e, -1.0, y_ap, ALU.add, ALU.max)
    else:
        t = pool.tile(shape, FP32, tag="elu_t")
        nc.vector.scalar_tensor_tensor(t, e, -1.0, y_ap, ALU.add, ALU.max)
        nc.vector.tensor_scalar_mul(out_ap, t, float(scale))


def build_nc(T=T_FULL):
    """Build the SPMD Bass program for one core (batch shard of 32)."""
    nc = bacc.Bacc("TRN2", target_bir_lowering=False, debug=False)

    d = {}
    d["action"] = nc.dram_tensor("action", [B, T, A], FP32, kind="ExternalInput").ap()
    d["osc"] = nc.dram_tensor("osc", [B, T, OSC_HALF], FP32, kind="ExternalInput").ap()
    d["motion"] = nc.dram_tensor("motion_state", [B, DM], FP32, kind="ExternalInput").ap()
    d["robot"] = nc.dram_tensor("robot_state", [B, DR], FP32, kind="ExternalInput").ap()
    d["mu"] = nc.dram_tensor("mu", [B, A], FP32, kind="ExternalInput").ap()
    d["mean"] = nc.dram_tensor("mean", [B, A], FP32, kind="ExternalInput").ap()
    d["Wm"] = nc.dram_tensor("Wm", [DM, U], FP32, kind="ExternalInput").ap()
    d["bm"] = nc.dram_tensor("bm", [U], FP32, kind="ExternalInput").ap()
    d["Wr"] = nc.dram_tensor("Wr", [DR, U], FP32, kind="ExternalInput").ap()
    d["br"] = nc.dram_tensor("br", [U], FP32, kind="ExternalInput").ap()
    d["Wc"] = nc.dram_tensor("Wc", [2 * U, U], FP32, kind="ExternalInput").ap()
    d["bc"] = nc.dram_tensor("bc", [U], FP32, kind="ExternalInput").ap()
    d["Wor"] = nc.dram_tensor("Wor", [OSC_HALF, OSC_HALF], FP32, kind="ExternalInput").ap()
    d["bor"] = nc.dram_tensor("bor", [OSC_HALF], FP32, kind="ExternalInput").ap()
    d["boi"] = nc.dram_tensor("boi", [OSC_HALF], FP32, kind="ExternalInput").ap()
    d["Wl"] = nc.dram_tensor("Wl", [A + 2 * OSC_HALF, U4], FP32, kind="ExternalInput").ap()
    d["bl"] = nc.dram_tensor("bl", [U4], FP32, kind="ExternalInput").ap()
    d["Ul"] = nc.dram_tensor("Ul", [U, U4], FP32, kind="ExternalInput").ap()
    d["Wo"] = nc.dram_tensor("Wo", [U, 1], FP32, kind="ExternalInput").ap()
    d["bo"] = nc.dram_tensor("bo", [1], FP32, kind="ExternalInput").ap()
    d["out"] = nc.dram_tensor("out", [B, 1], FP32, kind="ExternalOutput").ap()

    with tile.TileContext(nc) as tc, ExitStack() as ctx:
        _build_body(ctx, tc, T, d)
    nc.finalize()
    return nc


def _build_body(ctx, tc, T, d):
    nc = tc.nc
    CH = min(128, T)        # steps per production chunk
    assert T % CH == 0
    NCH = T // CH

    consts = ctx.enter_context(tc.tile_pool(name="consts", bufs=1))
    weights = ctx.enter_context(tc.tile_pool(name="weights", bufs=1))
    state = ctx.enter_context(tc.tile_pool(name="state", bufs=1))
    stage = ctx.enter_context(tc.tile_pool(name="stage", bufs=2))
    scratch = ctx.enter_context(tc.tile_pool(name="scratch", bufs=3))
    # PSUM budget (8 banks): ptrans 2 + pscan 1 + pmm 1 + pz 2x2 = 8
    ptrans = ctx.enter_context(tc.tile_pool(name="ptrans", bufs=2, space="PSUM"))
    pscan = ctx.enter_context(tc.tile_pool(name="pscan", bufs=1, space="PSUM"))
    pmm = ctx.enter_context(tc.tile_pool(name="pmm", bufs=1, space="PSUM"))
    pz_pool = ctx.enter_context(tc.tile_pool(name="pz", bufs=1, space="PSUM"))

    ident = consts.tile([128, 128], FP32)
    make_identity(nc, ident)
    ones_r = consts.tile([1, B], FP32)
    nc.vector.memset(ones_r, 1.0)
    identb = consts.tile([B, B], BF16)
    nc.vector.tensor_copy(identb, ident[0:B, 0:B])

    # ---------------- weights to SBUF (bf16, tanh-fold scaled) -------------
    # ulw[k]: Ul[128k:128k+128, :] col-scaled, bf16 — the scan's MOVING rhs.
    ulw = [weights.tile([128, U4], BF16, tag=f"ul_{k}", name=f"ul_{k}")
           for k in range(2)]
    for k in range(2):
        ust = stage.tile([128, U4], FP32, tag="ulst", name=f"ulst{k}")
        nc.sync.dma_start(out=ust, in_=d["Ul"][128 * k:128 * (k + 1), :])
        for g in range(4):
            nc.vector.tensor_scalar_mul(ulw[k][:, 256 * g:256 * (g + 1)],
                                        ust[:, 256 * g:256 * (g + 1)],
                                        UL_SCALE[g])

    # fused bias blEff = bl + elu(boi) @ Wl[96:160, :]  (fp32, full 1024)
    boi_sb = scratch.tile([OSC_HALF, 1], FP32)
    nc.sync.dma_start(out=boi_sb, in_=d["boi"].rearrange("(p one) -> p one", one=1))
    eboi = scratch.tile([OSC_HALF, 1], FP32)
    _elu(nc, scratch, eboi, boi_sb, [OSC_HALF, 1])
    wl_hi = scratch.tile([OSC_HALF, U4], FP32)
    nc.sync.dma_start(out=wl_hi, in_=d["Wl"][XROWS:XROWS + OSC_HALF, :])
    bl_sb = scratch.tile([1, U4], FP32)
    nc.sync.dma_start(out=bl_sb, in_=d["bl"].rearrange("(one n) -> one n", one=1))
    bleff = scratch.tile([1, U4], FP32)
    for half in range(2):
        p_bl = pmm.tile([1, 512], FP32, tag="mm", name=f"p_bl{half}")
        nc.tensor.matmul(p_bl, eboi, wl_hi[:, 512 * half:512 * (half + 1)],
                         start=True, stop=True)
        nc.vector.tensor_add(bleff[:, 512 * half:512 * (half + 1)], p_bl,
                             bl_sb[:, 512 * half:512 * (half + 1)])

    # wlb [97, 1024] bf16: rows 0:64 inp2-part of Wl, 64:96 action part,
    # row 96 = fused bias; cols scaled by WL_SCALE — the proj MOVING rhs.
    wlb = weights.tile([XK, U4], BF16, name="wlb")
    wlst = stage.tile([XK, U4], FP32, tag="wlst", name="wlst", bufs=1)
    nc.sync.dma_start(out=wlst[0:OSC_HALF, :], in_=d["Wl"][A:A + OSC_HALF, :])
    nc.sync.dma_start(out=wlst[OSC_HALF:XROWS, :], in_=d["Wl"][0:A, :])
    nc.vector.tensor_copy(wlst[XROWS:XK, :], bleff)
    for g in range(4):
        nc.vector.tensor_scalar_mul(wlb[:, 256 * g:256 * (g + 1)],
                                    wlst[:, 256 * g:256 * (g + 1)], WL_SCALE[g])

    # [Wor; bor] [65, 64] bf16 — inp2 projection stationary
    worb_f = scratch.tile([OSC_HALF + 1, OSC_HALF], FP32)
    nc.sync.dma_start(out=worb_f[0:OSC_HALF, :], in_=d["Wor"])
    nc.sync.dma_start(out=worb_f[OSC_HALF:OSC_HALF + 1, :],
                      in_=d["bor"].rearrange("(one n) -> one n", one=1))
    worb = weights.tile([OSC_HALF + 1, OSC_HALF], BF16)
    nc.vector.tensor_copy(worb, worb_f)

    # [Wm; bm] chunks [65, 128]
    wmb = [weights.tile([DM + 1, 128], FP32, tag=f"wm_{c}", name=f"wm_{c}") for c in range(2)]
    for c in range(2):
        nc.sync.dma_start(out=wmb[c][0:DM, :], in_=d["Wm"][:, 128 * c:128 * (c + 1)])
        nc.sync.dma_start(out=wmb[c][DM:DM + 1, :],
                          in_=d["bm"].rearrange("(one n) -> one n", one=1)[:, 128 * c:128 * (c + 1)])
    # Wr chunks [128,128] + br rows [1,128]
    wrb = [weights.tile([DR, 128], FP32, tag=f"wr_{c}", name=f"wr_{c}") for c in range(2)]
    brb = [weights.tile([1, 128], FP32, tag=f"br_{c}", name=f"br_{c}") for c in range(2)]
    for c in range(2):
        nc.sync.dma_start(out=wrb[c], in_=d["Wr"][:, 128 * c:128 * (c + 1)])
        nc.sync.dma_start(out=brb[c],
                          in_=d["br"].rearrange("(one n) -> one n", one=1)[:, 128 * c:128 * (c + 1)])
    # Wc chunks [128,128] (4 k-chunks x 2 m-chunks) + bc rows
    wcb = [[weights.tile([128, 128], FP32, tag=f"wc_{k}_{c}", name=f"wc_{k}_{c}") for c in range(2)]
           for k in range(4)]
    bcb = [weights.tile([1, 128], FP32, tag=f"bc_{c}", name=f"bc_{c}") for c in range(2)]
    for k in range(4):
        for c in range(2):
            nc.sync.dma_start(out=wcb[k][c],
                              in_=d["Wc"][128 * k:128 * (k + 1), 128 * c:128 * (c + 1)])
    for c in range(2):
        nc.sync.dma_start(out=bcb[c],
                          in_=d["bc"].rearrange("(one n) -> one n", one=1)[:, 128 * c:128 * (c + 1)])
    # Wo chunks [128,1] scaled 0.5 (Hmax = 2*hmax), bo [1,1]
    wob = [weights.tile([128, 1], FP32, tag=f"wo_{c}", name=f"wo_{c}") for c in range(2)]
    for c in range(2):
        st = scratch.tile([128, 1], FP32, tag="wstage1")
        nc.sync.dma_start(out=st, in_=d["Wo"][128 * c:128 * (c + 1), :])
        nc.vector.tensor_scalar_mul(wob[c], st, 0.5)
    bob = weights.tile([1, 1], FP32)
    nc.sync.dma_start(out=bob, in_=d["bo"].rearrange("(one n) -> one n", one=1))

    # muT/meanT [32a, 32b] via PE transpose
    mu_sb = scratch.tile([B, A], FP32)
    mean_sb = scratch.tile([B, A], FP32)
    nc.sync.dma_start(out=mu_sb, in_=d["mu"])
    nc.sync.dma_start(out=mean_sb, in_=d["mean"])
    muT = consts.tile([A, B], FP32)
    meanT = consts.tile([A, B], FP32)
    for src, dst in ((mu_sb, muT), (mean_sb, meanT)):
        pt = ptrans.tile([A, B], FP32, tag="pt", name="pt_mu")
        nc.tensor.transpose(pt, src, ident[0:B, 0:B])
        nc.vector.tensor_copy(dst, pt)

    # ---------------- xT: [97, T*32] bf16 feature-major input --------------
    # col = t*32 + b; rows 0:64 inp2, 64:96 scaled action, row 96 ones.
    xT = state.tile([XK, T * B], BF16)
    nc.vector.memset(xT[XROWS:XK, :], 1.0)
    if os.environ.get("KERNEL_SKIP_PRE"):
        nc.vector.memset(xT[0:XROWS, :], 0.01)
    else:
        _produce_xT(nc, tc, T, CH, NCH, d, stage, scratch, ptrans, pmm,
                    ident, muT, meanT, worb, xT)

    # ---------------- H0 = 2*h0, C2_0 = 2*c0 ----------------
    motT = scratch.tile([DM + 1, B], FP32)
    pt = ptrans.tile([DM, B], FP32, tag="pt", name="pt_mot")
    mot_sb = scratch.tile([B, DM], FP32)
    nc.sync.dma_start(out=mot_sb, in_=d["motion"])
    nc.tensor.transpose(pt, mot_sb, ident[0:B, 0:B])
    nc.vector.tensor_copy(motT[0:DM, :], pt)
    nc.vector.memset(motT[DM:DM + 1, :], 1.0)

    robT = scratch.tile([DR, B], FP32)
    pt = ptrans.tile([DR, B], FP32, tag="pt", name="pt_rob")
    rob_sb = scratch.tile([B, DR], FP32)
    nc.sync.dma_start(out=rob_sb, in_=d["robot"])
    nc.tensor.transpose(pt, rob_sb, ident[0:B, 0:B])
    nc.vector.tensor_copy(robT, pt)

    p_ms = pmm.tile([128, 2 * B], FP32, tag="mm", name="p_ms")
    for c in range(2):
        nc.tensor.matmul(p_ms[:, B * c:B * (c + 1)], wmb[c], motT,
                         start=True, stop=True)
    msT = scratch.tile([128, 2 * B], FP32, tag="msT")
    _elu(nc, scratch, msT, p_ms, [128, 2 * B])

    p_rs = pmm.tile([128, 2 * B], FP32, tag="mm", name="p_rs")
    for c in range(2):
        sl = p_rs[:, B * c:B * (c + 1)]
        nc.tensor.matmul(sl, wrb[c], robT, start=True, stop=False)
        nc.tensor.matmul(sl, brb[c], ones_r, start=False, stop=True)
    rsT = scratch.tile([128, 2 * B], FP32, tag="rsT")
    _elu(nc, scratch, rsT, p_rs, [128, 2 * B])

    p_st = pmm.tile([128, 2 * B], FP32, tag="mm", name="p_st")
    for c in range(2):
        sl = p_st[:, B * c:B * (c + 1)]
        nc.tensor.matmul(sl, wcb[0][c], msT[:, 0:B], start=True, stop=False)
        nc.tensor.matmul(sl, wcb[1][c], msT[:, B:2 * B], start=False, stop=False)
        nc.tensor.matmul(sl, wcb[2][c], rsT[:, 0:B], start=False, stop=False)
        nc.tensor.matmul(sl, wcb[3][c], rsT[:, B:2 * B], start=False, stop=False)
        nc.tensor.matmul(sl, bcb[c], ones_r, start=False, stop=True)

    # h0f [128, (k b)] feature-major; Hfm[0] = 2*h0f directly (bf16).
    h0f = scratch.tile([128, 2 * B], FP32, tag="h0f")
    _elu(nc, scratch, h0f, p_st, [128, 2 * B])
    Hfm = [state.tile([128, 2 * B], BF16, tag=f"Hfm{p}", name=f"Hfm{p}")
           for p in range(2)]
    nc.vector.tensor_scalar_mul(Hfm[0], h0f, 2.0)
    # C2_bm [32, 256] = 2*h0 batch-major via 2 transposes
    ptc = pmm.tile([B, U], FP32, tag="mm", name="ptc")
    for k in range(2):
        nc.tensor.transpose(ptc[:, 128 * k:128 * (k + 1)],
                            h0f[:, B * k:B * (k + 1)], ident)
    C2 = state.tile([B, U], FP32, name="C2")
    nc.vector.tensor_scalar_mul(C2, ptc, 2.0)

    Hring = state.tile([B, 4 * U], BF16, name="Hring")     # 4 slots, slot-major
    Hmax = state.tile([B, U], FP32, name="Hmax")
    nc.vector.memset(Hmax, -1e30)

    # ---------------- the scan ----------------
    gates = ctx.enter_context(tc.tile_pool(name="gates", bufs=2))
    T_SCAN = 0 if os.environ.get("KERNEL_SKIP_SCAN") else T
    assert T_SCAN % 4 == 0 or T_SCAN == 0
    # pz[p]: 2 steps' z stacked on partitions [2*32, 1024]; 2 banks each.
    pz = [pz_pool.tile([2 * B, U4], FP32, tag=f"pz{p}", name=f"pz{p}")
          for p in range(2)]

    def proj(g):
        # x-projection for steps 2g, 2g+1 into pz[g%2]: one stationary of
        # 64 x-columns -> out rows (s,b) stacked; starts the accumulation.
        pzN = pz[g % 2]
        xs = xT[:, 2 * B * g:2 * B * (g + 1)]
        for h in range(2):
            nc.tensor.matmul(pzN[:, 512 * h:512 * (h + 1)], xs,
                             wlb[:, 512 * h:512 * (h + 1)],
                             start=True, stop=False, skip_group_check=True)

    if T_SCAN:
        proj(0)
    amort = (T_SCAN % 4 == 0)
    for t in range(T_SCAN):
        s = t % 2
        par = (t // 2) % 2
        pzC = pz[par]
        if s == 1 and t + 2 < T_SCAN:
            proj(t // 2 + 1)
        # z_t += Ul^T H: H stationary (bf16), Ul moving
        for k in range(2):
            lh = Hfm[t % 2][:, B * k:B * (k + 1)]
            for h in range(2):
                nc.tensor.matmul(pzC[B * s:B * (s + 1), 512 * h:512 * (h + 1)],
                                 lh, ulw[k][:, 512 * h:512 * (h + 1)],
                                 start=False, stop=(k == 1),
                                 skip_group_check=True)
        t_all = gates.tile([B, U4], BF16, tag="tall")
        nc.scalar.activation(t_all, pzC[B * s:B * (s + 1), :], AF.Tanh)
        ti = t_all[:, 0:256]
        tf = t_all[:, 256:512]
        tg = t_all[:, 512:768]
        to = t_all[:, 768:1024]
        # B2 = (ti + 1) * tg ; A2 = (tf + 1) * C2 ; C2' = 0.5*A2 + B2
        B2 = gates.tile([B, U], BF16, tag="B2")
        nc.vector.scalar_tensor_tensor(B2, ti, 1.0, tg, ALU.add, ALU.mult)
        A2 = gates.tile([B, U], FP32, tag="A2")
        nc.vector.scalar_tensor_tensor(A2, tf, 1.0, C2, ALU.add, ALU.mult)
        nc.vector.scalar_tensor_tensor(C2, A2, 0.5, B2, ALU.mult, ALU.add)
        # TC = tanh(0.5 * C2') ; H = (to + 1) * TC -> ring slot
        TC = gates.tile([B, U], BF16, tag="TC")
        nc.scalar.activation(TC, C2, AF.Tanh, scale=0.5)
        slot = Hring[:, U * (t % 4):U * (t % 4 + 1)]
        nc.vector.scalar_tensor_tensor(slot, to, 1.0, TC, ALU.add, ALU.mult)
        if amort:
            if t % 4 == 3:
                rmax = gates.tile([B, U], FP32, tag="rmax")
                nc.vector.tensor_reduce(
                    rmax, Hring.rearrange("p (s u) -> p u s", s=4),
                    axis=mybir.AxisListType.X, op=ALU.max)
                nc.vector.tensor_max(Hmax, Hmax, rmax)
        else:
            nc.vector.tensor_max(Hmax, Hmax, slot)
        if t + 1 < T_SCAN:
            # H^T for next step: 2 PE transposes + 1 copy
            ptr = pscan.tile([128, 2 * B], BF16, tag="ptr", name="ptr")
            for k in range(2):
                nc.tensor.transpose(ptr[:, B * k:B * (k + 1)],
                                    slot[:, 128 * k:128 * (k + 1)], identb)
            nc.vector.tensor_copy(Hfm[(t + 1) % 2], ptr)

    # ---------------- output ----------------
    # Hmax^T [128, 2B] feature-major via 2 transposes
    pto = ptrans.tile([128, 2 * B], FP32, tag="pt", name="pt_out")
    for k in range(2):
        nc.tensor.transpose(pto[:, B * k:B * (k + 1)],
                            Hmax[:, 128 * k:128 * (k + 1)], ident[0:B, 0:B])
    hmf = scratch.tile([128, 2 * B], FP32, tag="hmf")
    nc.vector.tensor_copy(hmf, pto)
    p_out = pmm.tile([1, B], FP32, tag="mm", name="p_out")
    nc.tensor.matmul(p_out, bob, ones_r, start=True, stop=False)
    nc.tensor.matmul(p_out, wob[0], hmf[:, 0:B], start=False, stop=False)
    nc.tensor.matmul(p_out, wob[1], hmf[:, B:2 * B], start=False, stop=True)
    out_sb = scratch.tile([1, B], FP32)
    _elu(nc, scratch, out_sb, p_out, [1, B])
    nc.sync.dma_start(out=d["out"].rearrange("b one -> one b"), in_=out_sb)


def _produce_xT(nc, tc, T, CH, NCH, d, stage, scratch, ptrans, pmm,
                ident, muT, meanT, worb, xT):
    """Fill xT[0:96] from action/osc: DMA -> PE transpose -> affine/elu."""
    # xT views: cols (t b) -> [p, t, b] and [p, b, t]
    xTa = xT[OSC_HALF:XROWS, :].rearrange("p (t b) -> p t b", b=B)
    xTo = xT[0:OSC_HALF, :].rearrange("p (t b) -> p b t", b=B)
    for j in range(NCH):
        # staging slab [CH t, 32 b x (64 osc | 32 act)] f32
        ao = stage.tile([CH, B * XROWS], FP32, tag="ao", name=f"ao{j}")
        aov = ao.rearrange("p (b f) -> p b f", f=XROWS)
        nc.sync.dma_start(
            out=aov[:, :, OSC_HALF:XROWS],
            in_=d["action"][:, CH * j:CH * (j + 1), :].rearrange("b t a -> t b a"))
        nc.sync.dma_start(
            out=aov[:, :, 0:OSC_HALF],
            in_=d["osc"][:, CH * j:CH * (j + 1), :].rearrange("b t o -> t b o"))
        # osc feature-major staging for the bulk Wor matmul (b-major cols)
        oT = stage.tile([OSC_HALF + 1, B * CH], BF16, tag="oT", name=f"oT{j}")
        nc.vector.memset(oT[OSC_HALF:OSC_HALF + 1, :], 1.0)
        for b in range(B):
            pt = ptrans.tile([XROWS, CH], FP32, tag="pt", name="pt_ao")
            nc.tensor.transpose(pt, aov[:, b, :], ident[0:CH, 0:CH])
            # action rows: affine (mu, mean) evac -> xT[64:96]
            nc.vector.tensor_scalar(xTa[:, CH * j:CH * (j + 1), b],
                                    pt[OSC_HALF:XROWS, :],
                                    muT[:, b:b + 1], meanT[:, b:b + 1],
                                    ALU.mult, ALU.add)
            # osc rows: copy evac (bf16)
            nc.vector.tensor_copy(oT[0:OSC_HALF, CH * b:CH * (b + 1)],
                                  pt[0:OSC_HALF, :])
        # inp2 = elu(osc @ Wor + bor) in blocks of <=512 cols
        NB = (B * CH + 511) // 512
        BW = (B * CH) // NB
        assert BW % CH == 0
        for n in range(NB):
            pw = pmm.tile([OSC_HALF, BW], FP32, tag="mm", name="pw")
            nc.tensor.matmul(pw, worb, oT[:, BW * n:BW * (n + 1)],
                             start=True, stop=True)
            # elu -> xT[0:64] with (b,t) -> col (t*32+b) scatter
            b0 = (BW * n) // CH
            nb = BW // CH
            dst = xTo[:, b0:b0 + nb, CH * j:CH * (j + 1)]
            pwv = pw.rearrange("p (b t) -> p b t", t=CH)
            m = scratch.tile([OSC_HALF, BW], FP32, tag="elu_m2")
            nc.vector.tensor_scalar_min(m, pw, 0.0)
            e = scratch.tile([OSC_HALF, BW], FP32, tag="elu_e2")
            nc.scalar.activation(e, m, AF.Exp)
            ev = e.rearrange("p (b t) -> p b t", t=CH)
            nc.vector.scalar_tensor_tensor(dst, ev, -1.0, pwv, ALU.add, ALU.max)


# ------------------------------------------------------------------
# host-side entry point
# ------------------------------------------------------------------
_CACHE = {}
_EXEC_CACHE = {}


def _shard_inputs(inputs, T):
    """Split batch across cores; replicate weights."""
    batch_keys = ["action", "osc", "motion_state", "robot_state", "mu", "mean"]
    wkeys = ["Wm", "bm", "Wr", "br", "Wc", "bc", "Wor", "bor", "boi",
             "Wl", "bl", "Ul", "Wo", "bo"]
    in_maps = []
    for i in range(NCORES):
        s = slice(B * i, B * (i + 1))
        m = {}
        for k in batch_keys:
            v = np.asarray(inputs[k], dtype=np.float32)[s]
            if k == "action":
                v = v[:, :T]
            elif k == "osc":
                # only the first half of the osc features is ever read
                v = v[:, :T, :OSC_HALF]
            m[k] = np.ascontiguousarray(v)
        for k in wkeys:
            m[k] = np.ascontiguousarray(np.asarray(inputs[k], dtype=np.float32))
        in_maps.append(m)
    return in_maps


def _build_exec(T):
    """Compile once: return a callable taking the global (concat) inputs."""
    import jax
    from jax.sharding import Mesh, PartitionSpec
    from jax.experimental.shard_map import shard_map
    from concourse import bass2jax

    if T not in _CACHE:
        _CACHE[T] = build_nc(T)
    nc = _CACHE[T]
    bass2jax.install_neuronx_cc_hook()

    part_name = nc.partition_id_tensor.name if nc.partition_id_tensor else None
    in_names, out_names, out_avals, zero_outs = [], [], [], []
    for alloc in nc.m.functions[0].allocations:
        if not isinstance(alloc, mybir.MemoryLocationSet):
            continue
        name = alloc.memorylocations[0].name
        if alloc.kind == "ExternalInput":
            if name != part_name:
                in_names.append(name)
        elif alloc.kind == "ExternalOutput":
            out_names.append(name)
            shape = tuple(alloc.tensor_shape)
            dtype = mybir.dt.np(alloc.dtype)
            out_avals.append(jax.core.ShapedArray(shape, dtype))
            zero_outs.append(np.zeros(shape, dtype))
    n_params = len(in_names)
    all_names = in_names + out_names
    if part_name is not None:
        all_names = all_names + [part_name]

    def _body(*args):
        operands = list(args)
        if part_name is not None:
            operands.append(bass2jax.partition_id_tensor())
        outs = bass2jax._bass_exec_p.bind(
            *operands,
            out_avals=tuple(out_avals),
            in_names=tuple(all_names),
            out_names=tuple(out_names),
            lowering_input_output_aliases=(),
            sim_require_finite=True,
            sim_require_nnan=True,
            nc=nc,
        )
        return tuple(outs)

    devices = jax.devices()[:NCORES]
    mesh = Mesh(np.asarray(devices), ("core",))
    n_outs = len(out_names)
    sharded = jax.jit(shard_map(
        _body, mesh=mesh,
        in_specs=(PartitionSpec("core"),) * (n_params + n_outs),
        out_specs=(PartitionSpec("core"),) * n_outs,
        check_rep=False))
    concat_zero = [np.zeros((NCORES * z.shape[0], *z.shape[1:]), z.dtype)
                   for z in zero_outs]

    def run(global_in: dict):
        args = [global_in[name] for name in in_names] + concat_zero
        outs = sharded(*args)
        return {name: np.asarray(outs[i]) for i, name in enumerate(out_names)}

    return run


def _global_inputs(inputs, T):
    """Concat-across-cores input dict (minimal copying)."""
    g = {}
    act = np.asarray(inputs["action"], dtype=np.float32)
    g["action"] = np.ascontiguousarray(act[:, :T])
    osc = np.asarray(inputs["osc"], dtype=np.float32)
    g["osc"] = np.ascontiguousarray(osc[:, :T, :OSC_HALF])
    for k in ["motion_state", "robot_state", "mu", "mean"]:
        g[k] = np.ascontiguousarray(np.asarray(inputs[k], dtype=np.float32))
    for k in ["Wm", "bm", "Wr", "br", "Wc", "bc", "Wor", "bor", "boi",
              "Wl", "bl", "Ul", "Wo", "bo"]:
        w = np.ascontiguousarray(np.asarray(inputs[k], dtype=np.float32))
        g[k] = np.concatenate([w] * NCORES, axis=0)
    return g


def kernel(**inputs) -> np.ndarray:
    T = int(np.asarray(inputs["action"]).shape[1])
    if T not in _EXEC_CACHE:
        _EXEC_CACHE[T] = _build_exec(T)
    out = _EXEC_CACHE[T](_global_inputs(inputs, T))["out"]
    return out.reshape(B_FULL, 1).astype(np.float32)


if __name__ == "__main__":
    nc = build_nc(128)
    print("built ok")
